# revision 1
# baseline (speedup 1.0000x reference)
"""Trainium2 Bass kernel for batched per-item GRU cell.

Problem: nn_GRU_Cell — B=16, N=207 independent items, each with its own
C=64 -> 3H=192 weight matrices (Wx, Wh).  All ops are per-(b,n):

    xW          = x @ Wx                      [1, 192]
    r           = sigmoid(xW_r + h @ Wh_r + b_r)
    z           = sigmoid(xW_z + h @ Wh_z + b_z)
    hc          = tanh  (xW_c + (r*h) @ Wh_c + b_c)
    h_new       = (1 - z) * h + z * hc

Strategy (per core, items sharded 3312 -> 8 x 414):
  * Weights are the DMA bottleneck (~41MB/core).  Stream them once.
  * Per item, weights become the PE *stationary* operand, K-stacked:
      S_rz = [Wx[:, 0:128] ; Wh[:, 0:128]]  (K=128, M=128)
      S_c  = [Wx[:,128:192]; Wh[:,128:192]] (K=128, M=64)
    and the moving operand is a single column:
      rz-pass: [x ; h]         -> psum_rz[:, item]  (r rows 0:64, z rows 64:128)
      c-pass : [x ; r*h]       -> psum_c [64:128, item]  (folds xW_c in!)
    Outputs land as dense PSUM columns [j, item] -> cheap eviction.
  * Activations / bias are transposed to [j, items] with f32 PE
    transposes; the whole epilogue runs on DVE/ACT at partition
    offsets 64:128 so no cross-partition moves are ever needed.
"""

import numpy as np

import concourse.bass as bass
import concourse.mybir as mybir
import concourse.tile as tile
from concourse import bacc
from concourse.bass_utils import run_bass_kernel_spmd
from concourse.masks import make_identity

F32 = mybir.dt.float32
BF16 = mybir.dt.bfloat16

B, N, C, H = 16, 207, 64, 64
J = 3 * H                  # 192
ITEMS = B * N              # 3312
NCORES = 8
PER = ITEMS // NCORES      # 414
# Small chunks keep the PE's per-chunk DMA-wait gaps under the ~3.4us HAM
# re-throttle window so matmuls stay at 2.4 GHz.  The short LAST chunk
# shrinks the serial drain after the final weight transfer.
CHUNKS = [48] * 8 + [30]   # sum = 414
NCHUNK = len(CHUNKS)
GMAX = max(CHUNKS)

AF = mybir.ActivationFunctionType


def build_nc(wdt=F32, mdt=F32):
    """Build the per-core Bass program.

    wdt: dtype of the streamed weights (DMA volume / LDW speed).
    mdt: dtype of the moving operand columns (must pair with wdt for PE).
    """
    # Bacc (not raw Bass): its compile() runs move_matmul_waits_to_ldweights
    # + generate_event_semaphores, which split multi-waits down to the 1-wait
    # ISA limit of PE instructions.
    nc = bacc.Bacc(None)
    # x|h and Wx|Wh are concatenated host-side: one DMA writer per SBUF
    # tile keeps every PE instruction's semaphore fan-in <= 2 (ISA limit),
    # and a single weight DMA spans all 128 partitions = all 16 SDMA engines.
    aux_d = nc.declare_dram_parameter("aux", [PER, 2 * C + J], F32, isOutput=False)
    # weights arrive host-pre-transposed to per-chunk [c, item, j] blocks
    # (flattened): the per-chunk DMA reads one ~35KB contiguous run per
    # partition instead of 768B rows
    w_d = nc.declare_dram_parameter("wxh", [PER * 2 * C * J], wdt,
                                    isOutput=False)
    out_d = nc.declare_dram_parameter("out", [PER, H], F32, isOutput=True)

    cast_rhs = mdt != F32

    with tile.TileContext(nc) as tc:
        with (
            tc.tile_pool(name="const", bufs=1) as cpool,
            tc.tile_pool(name="w", bufs=3) as wpool,
            tc.tile_pool(name="stage", bufs=2) as spool,
            tc.tile_pool(name="act", bufs=2) as apool,
            tc.tile_pool(name="prep", bufs=2, space="PSUM") as prep_pool,
            tc.tile_pool(name="prz", bufs=2, space="PSUM") as prz_pool,
            tc.tile_pool(name="pc", bufs=2, space="PSUM") as pc_pool,
            tc.tile_pool(name="pt", bufs=2, space="PSUM") as pt_pool,
        ):
            ident = cpool.tile([128, 128], F32)
            make_identity(nc, ident[:])

            s = 0
            woff = 0
            for k in range(NCHUNK):
                G = CHUNKS[k]

                # ---- stream this chunk's weights -------------------------
                # w[c(0:64) | c(64:128), item, j] = [Wx ; Wh]
                # alternate the two HWDGE queues so one queue's issue/wait
                # overhead hides behind the other queue's in-flight transfer
                w = wpool.tile([128, GMAX, J], wdt, tag="w")
                wq = nc.sync if k % 2 == 0 else nc.scalar
                wq.dma_start(
                    out=w[:, 0:G, :],
                    in_=w_d[woff:woff + 128 * G * J].rearrange(
                        "(c g j) -> c g j", c=128, g=G),
                )

                # ---- transpose x/h and bias to [j, items] ----------------
                # one aux DMA carries x|h (cols 0:128) and b (cols 128:320);
                # issued from ACT so the SP queue carries nothing but the
                # back-to-back weight stream
                aux_full = spool.tile([GMAX, 2 * C + J], F32, tag="aux")
                aux = aux_full[0:G]
                nc.gpsimd.dma_start(out=aux[:], in_=aux_d[s:s + G])
                txh = aux[:, 0:128]
                tb = aux[:, 128:128 + J]
                p_xh = prep_pool.tile([128, G], F32, tag="prep")
                nc.tensor.transpose(p_xh[:], txh[:], ident[0:G, 0:G])
                # xh: rows 0:64 = x.T, rows 64:128 = h.T   (f32 master copy)
                xh = apool.tile([128, G], F32, tag="xh")
                nc.scalar.activation(xh[:], p_xh[:], AF.Copy)
                if cast_rhs:
                    xh_m = apool.tile([128, G], mdt, tag="xh_m")
                    nc.vector.tensor_copy(xh_m[:], xh[:])
                else:
                    xh_m = xh

                p_b = prep_pool.tile([128, G], F32, tag="prep")
                nc.tensor.transpose(p_b[:], tb[:, 0:128], ident[0:G, 0:G])
                b_rz = apool.tile([128, G], F32, tag="b_rz")
                nc.scalar.activation(b_rz[:], p_b[:], AF.Copy)
                p_bc = prep_pool.tile([128, G], F32, tag="prep")
                nc.tensor.transpose(p_bc[0:64, :], tb[:, 128:192], ident[0:G, 0:G])
                b_c = apool.tile([128, G], F32, tag="b_c")
                nc.scalar.activation(b_c[0:64, :], p_bc[0:64, :], AF.Copy)

                # ---- pass 1: per-item rz matmul --------------------------
                psum_rz = prz_pool.tile([128, G], F32, tag="rz")
                for g in range(G):
                    nc.tensor.matmul(
                        psum_rz[:, g:g + 1],
                        w[:, g, 0:128],
                        xh_m[:, g:g + 1],
                        start=True, stop=True,
                    )

                # ---- epilogue 1: r, z, and the c-pass moving columns -----
                t_rz = apool.tile([128, G], F32, tag="t_rz")
                nc.vector.tensor_add(t_rz[:], psum_rz[:], b_rz[:])
                # r evicted to rows 64:128 so r*h aligns with h there
                rs = apool.tile([128, G], F32, tag="rs")
                nc.scalar.activation(rs[64:128, :], t_rz[0:64, :], AF.Sigmoid)
                zs = apool.tile([128, G], F32, tag="zs")
                nc.scalar.activation(zs[64:128, :], t_rz[64:128, :], AF.Sigmoid)
                rhs2 = apool.tile([128, G], mdt, tag="rhs2")
                nc.vector.tensor_copy(rhs2[0:64, :], xh_m[0:64, :])
                nc.vector.tensor_mul(rhs2[64:128, :], rs[64:128, :], xh[64:128, :])

                # ---- pass 2: per-item c matmul (xW_c + (r*h) @ Wh_c) -----
                psum_c = pc_pool.tile([128, G], F32, tag="c")
                for g in range(G):
                    nc.tensor.matmul(
                        psum_c[0:64, g:g + 1],
                        w[:, g, 128:192],
                        rhs2[:, g:g + 1],
                        start=True, stop=True,
                    )

                # ---- epilogue 2: hc, h_new (h_new on partitions 64:128) --
                t_c = apool.tile([128, G], F32, tag="t_c")
                nc.vector.tensor_add(t_c[0:64, :], psum_c[0:64, :], b_c[0:64, :])
                # cross-offset ACT move puts hc on 64:128 where z and h live
                hc = apool.tile([128, G], F32, tag="hc")
                nc.scalar.activation(hc[64:128, :], t_c[0:64, :], AF.Tanh)
                zh = apool.tile([128, G], F32, tag="zh")
                nc.vector.tensor_mul(zh[64:128, :], zs[64:128, :], xh[64:128, :])
                zhc = apool.tile([128, G], F32, tag="zhc")
                nc.vector.tensor_mul(zhc[64:128, :], zs[64:128, :], hc[64:128, :])
                d = apool.tile([128, G], F32, tag="d")
                nc.vector.tensor_sub(d[64:128, :], xh[64:128, :], zh[64:128, :])
                hn = apool.tile([128, G], F32, tag="hn")
                nc.vector.tensor_add(hn[64:128, :], d[64:128, :], zhc[64:128, :])

                # ---- transpose back to [items, H] and store --------------
                p_t = pt_pool.tile([128, 64], F32, tag="t")
                nc.tensor.transpose(
                    p_t[0:G, :], hn[64:128, 0:G], ident[64:128, 64:128]
                )
                ot = spool.tile([G, 64], F32, tag="ot")
                nc.scalar.activation(ot[:], p_t[0:G, :], AF.Copy)
                # SWDGE store: its wait on ot must not stall either HWDGE
                # queue that streams weights
                nc.gpsimd.dma_start(out=out_d[s:s + G], in_=ot[:])

                s += G
                woff += 128 * G * J

    nc.compile()
    return nc


_CACHE = {}


def _get_nc(wdt, mdt):
    key = (wdt, mdt)
    if key not in _CACHE:
        _CACHE[key] = build_nc(wdt, mdt)
    return _CACHE[key]


def _shards(x, state, Wx, Wh, b, wdt_np):
    x2 = np.asarray(x, np.float32).reshape(ITEMS, C)
    h2 = np.asarray(state, np.float32).reshape(ITEMS, H)
    b2 = np.asarray(b, np.float32).reshape(ITEMS, J)
    aux2 = np.ascontiguousarray(np.concatenate([x2, h2, b2], axis=1))
    wx2 = np.asarray(Wx).reshape(ITEMS, C, J)
    wh2 = np.asarray(Wh).reshape(ITEMS, H, J)
    w2 = np.concatenate([wx2, wh2], axis=1).astype(wdt_np)
    w2 = w2.reshape(NCORES, PER, 2 * C, J)
    maps = []
    for i in range(NCORES):
        sl = slice(i * PER, (i + 1) * PER)
        # per chunk: [items, c, j] -> [c, item-in-chunk, j], flattened
        blocks = []
        s = 0
        for G in CHUNKS:
            blocks.append(w2[i, s:s + G].transpose(1, 0, 2).ravel())
            s += G
        maps.append({"aux": aux2[sl], "wxh": np.concatenate(blocks)})
    return maps


def kernel(x, state, Wx, Wh, b, _trace=False, _wdt=F32, _mdt=F32):
    import ml_dtypes
    wdt_np = np.float32 if _wdt == F32 else ml_dtypes.bfloat16
    nc = _get_nc(_wdt, _mdt)
    in_maps = _shards(x, state, Wx, Wh, b, wdt_np)
    res = run_bass_kernel_spmd(nc, in_maps, list(range(NCORES)), trace=_trace)
    out = np.concatenate([res.results[i]["out"] for i in range(NCORES)], axis=0)
    ret = out.reshape(B, N, 1, H).astype(np.float32)
    if _trace:
        return ret, res
    return ret



# revision 2
# speedup vs baseline: 3.3582x; 3.3582x over previous
"""Trainium2 Bass kernel for batched per-item GRU cell (bf16 PE pipeline).

Problem: nn_GRU_Cell — B=16, N=207 independent items, each with its own
C=64 -> 3H=192 weight matrices (Wx, Wh).  All ops are per-(b,n):

    xW          = x @ Wx                      [1, 192]
    r           = sigmoid(xW_r + h @ Wh_r + b_r)
    z           = sigmoid(xW_z + h @ Wh_z + b_z)
    hc          = tanh  (xW_c + (r*h) @ Wh_c + b_c)
    h_new       = (1 - z) * h + z * hc

Strategy (per core, items sharded 3312 -> 8 x 414):
  * Weights are both the DMA and the PE bottleneck.  They stream ONCE as
    bf16 (halves HBM traffic vs f32, and bf16 stationaries load with the
    PE's Fast-Weight-Load at ~2x; f32 matmuls would also double-pass).
    Accuracy: bf16 weights/activations with f32 PSUM accumulate gives
    rel-err ~1.8e-3 on this problem (gate is 2e-2).
  * Per item the weights are the PE *stationary* operand, K-stacked:
      S_rz       = [Wx[:, 0:128] ; Wh[:, 0:128]]    (K=128, M=128)
      S_c (pair) = [Wc_even | Wc_odd], Wc = [Wx[:,128:192]; Wh[:,128:192]]
    The c-weights of two adjacent items are packed into ONE 128-column
    stationary so every LDWEIGHTS is a full 128-column load (FWL-eligible)
    and two c-matmuls share one weight load.
  * Moving operands are single bf16 columns:
      rz: [x ; h]     -> psum_rz[:, g]  (r rows 0:64, z rows 64:128)
      c : [x ; r*h]   -> psum_c [:, g]  (even items' c in rows 0:64,
                                         odd items' c in rows 64:128)
  * All input transposition is done HOST-side: x/h/biases arrive as
    [feature, item] panels, so the kernel issues ZERO PE transposes and
    no ACT staging copies.  The output stays [H, items]; the host
    transposes it back.
  * The c-pass of chunk k is issued after the rz-pass of chunk k+1, so
    the PE never waits on the sigmoid/DVE producing the c moving operand.
"""

import numpy as np

import concourse.bass as bass
import concourse.mybir as mybir
import concourse.tile as tile
from concourse import bacc
from concourse.bass_utils import run_bass_kernel_spmd

F32 = mybir.dt.float32
BF16 = mybir.dt.bfloat16
AF = mybir.ActivationFunctionType

B, N, C, H = 16, 207, 64, 64
J = 3 * H                  # 192
ITEMS = B * N              # 3312
NCORES = 8
PER = ITEMS // NCORES      # 414
CHUNKS = [52] * 7 + [50]   # sum = 414; all even (c-pass pairs items)
NCHUNK = len(CHUNKS)
GMAX = max(CHUNKS)


def build_nc():
    nc = bacc.Bacc(None)
    # host-pre-transposed activation panels
    #   xhx  bf16 [128, 2*PER]: cols 0:PER = [x;h], cols PER:2PER = [x;0]
    #        (rows 64:128 of the second block get r*h written on-chip)
    #   auxf f32  [128, 3*PER]: block0 rows 64:128 = h, block1 = b_rz^T,
    #        block2 = b_c duplicated (even cols rows 0:64, odd rows 64:128)
    xhx_d = nc.declare_dram_parameter("xhx", [128, 2 * PER], BF16,
                                      isOutput=False)
    auxf_d = nc.declare_dram_parameter("auxf", [128, 3 * PER], F32,
                                       isOutput=False)
    # per-chunk blocks, each [c=128, G*192] flattened: G rz-stationaries
    # (128 cols each) then G/2 paired c-stationaries (128 cols each)
    w_d = nc.declare_dram_parameter("wxh", [PER * 2 * C * J], BF16,
                                    isOutput=False)
    out_d = nc.declare_dram_parameter("out", [64, PER], F32, isOutput=True)

    with tile.TileContext(nc) as tc:
        with (
            tc.tile_pool(name="const", bufs=1) as cpool,
            tc.tile_pool(name="w", bufs=3) as wpool,
            tc.tile_pool(name="ep", bufs=2) as epool,
            tc.tile_pool(name="prz", bufs=2, space="PSUM") as prz_pool,
            tc.tile_pool(name="pc", bufs=2, space="PSUM") as pc_pool,
        ):
            xhx = cpool.tile([128, 2 * PER], BF16)
            nc.scalar.dma_start(out=xhx[:], in_=xhx_d[:])
            auxf = cpool.tile([128, 3 * PER], F32)
            nc.scalar.dma_start(out=auxf[:], in_=auxf_d[:])
            hn = cpool.tile([128, PER], F32)

            h_f = auxf[:, 0:PER]               # rows 64:128 = h (f32)
            b_rz = auxf[:, PER:2 * PER]
            bc2 = auxf[:, 2 * PER:3 * PER]
            rhs2 = xhx[:, PER:2 * PER]         # rows 0:64 = x (bf16)

            # software pipeline state: chunk k's c-pass+epilogue runs after
            # chunk k+1's rz-pass so PE never stalls on the sigmoid chain
            pending = None

            def rz_pass(k, s, G):
                w = wpool.tile([128, GMAX * J], BF16, tag="w")
                wq = nc.sync if k % 2 == 0 else nc.scalar
                wq.dma_start(
                    out=w[:, 0:G * J],
                    in_=w_d[s * 128 * J:(s + G) * 128 * J].rearrange(
                        "(c v) -> c v", c=128),
                )
                psum_rz = prz_pool.tile([128, GMAX], F32, tag="rz")
                for g in range(G):
                    nc.tensor.matmul(
                        psum_rz[:, g:g + 1],
                        w[:, g * 128:(g + 1) * 128],
                        xhx[:, s + g:s + g + 1],
                        start=True, stop=True,
                    )
                # r/z and the c-pass moving columns
                t_rz = epool.tile([128, GMAX], F32, tag="t_rz")
                nc.vector.tensor_add(t_rz[:, 0:G], psum_rz[:, 0:G],
                                     b_rz[:, s:s + G])
                rs = epool.tile([128, GMAX], F32, tag="rs")
                nc.scalar.activation(rs[64:128, 0:G], t_rz[0:64, 0:G],
                                     AF.Sigmoid)
                zs = epool.tile([128, GMAX], F32, tag="zs")
                nc.scalar.activation(zs[64:128, 0:G], t_rz[64:128, 0:G],
                                     AF.Sigmoid)
                nc.vector.tensor_mul(rhs2[64:128, s:s + G], rs[64:128, 0:G],
                                     h_f[64:128, s:s + G])
                return w, zs

            def c_pass(k, s, G, w, zs):
                psum_c = pc_pool.tile([128, GMAX], F32, tag="c")
                cbase = G * 128
                for t in range(G // 2):
                    lw = w[:, cbase + t * 128:cbase + (t + 1) * 128]
                    nc.tensor.matmul(
                        psum_c[:, 2 * t:2 * t + 1], lw,
                        rhs2[:, s + 2 * t:s + 2 * t + 1],
                        start=True, stop=True,
                    )
                    nc.tensor.matmul(
                        psum_c[:, 2 * t + 1:2 * t + 2], lw,
                        rhs2[:, s + 2 * t + 1:s + 2 * t + 2],
                        start=True, stop=True,
                    )
                t_c = epool.tile([128, GMAX], F32, tag="t_c")
                nc.vector.tensor_add(t_c[:, 0:G], psum_c[:, 0:G],
                                     bc2[:, s:s + G])
                # even items' c sits in rows 0:64, odd items' in 64:128
                hc = epool.tile([128, GMAX], F32, tag="hc")
                nc.scalar.activation(hc[64:128, 0:G:2], t_c[0:64, 0:G:2],
                                     AF.Tanh)
                nc.scalar.activation(hc[64:128, 1:G:2], t_c[64:128, 1:G:2],
                                     AF.Tanh)
                # h_new = h + z*(hc - h)
                diff = epool.tile([128, GMAX], F32, tag="diff")
                nc.vector.tensor_sub(diff[64:128, 0:G], hc[64:128, 0:G],
                                     h_f[64:128, s:s + G])
                prod = epool.tile([128, GMAX], F32, tag="prod")
                nc.vector.tensor_mul(prod[64:128, 0:G], zs[64:128, 0:G],
                                     diff[64:128, 0:G])
                nc.vector.tensor_add(hn[64:128, s:s + G],
                                     h_f[64:128, s:s + G],
                                     prod[64:128, 0:G])
                nc.gpsimd.dma_start(out=out_d[:, s:s + G],
                                    in_=hn[64:128, s:s + G])

            s = 0
            for k in range(NCHUNK):
                G = CHUNKS[k]
                state = rz_pass(k, s, G)
                if pending is not None:
                    c_pass(*pending)
                pending = (k, s, G) + state
                s += G
            c_pass(*pending)

    nc.compile()
    return nc


_CACHE = {}


def _get_nc():
    if "nc" not in _CACHE:
        _CACHE["nc"] = build_nc()
    return _CACHE["nc"]


def _pack(x, state, Wx, Wh, b):
    import ml_dtypes
    BF = ml_dtypes.bfloat16
    x2 = np.asarray(x, np.float32).reshape(ITEMS, C)
    h2 = np.asarray(state, np.float32).reshape(ITEMS, H)
    b2 = np.asarray(b, np.float32).reshape(ITEMS, J)
    wx = np.asarray(Wx, np.float32).reshape(ITEMS, C, J)
    wh = np.asarray(Wh, np.float32).reshape(ITEMS, H, J)
    w2 = np.concatenate([wx, wh], axis=1).astype(BF)   # [ITEMS, 128, 192]
    maps = []
    for i in range(NCORES):
        sl = slice(i * PER, (i + 1) * PER)
        xi, hi, bi, wi = x2[sl], h2[sl], b2[sl], w2[sl]

        xhx = np.zeros((128, 2 * PER), BF)
        xhx[0:64, 0:PER] = xi.T
        xhx[64:128, 0:PER] = hi.T
        xhx[0:64, PER:2 * PER] = xi.T

        auxf = np.zeros((128, 3 * PER), np.float32)
        auxf[64:128, 0:PER] = hi.T
        auxf[:, PER:2 * PER] = bi[:, 0:128].T
        bc = bi[:, 128:192].T                    # [64, PER]
        auxf[0:64, 2 * PER + 0:3 * PER:2] = bc[:, 0::2]
        auxf[64:128, 2 * PER + 1:3 * PER:2] = bc[:, 1::2]

        blocks = []
        s = 0
        for G in CHUNKS:
            wc = wi[s:s + G]                               # [G, 128, 192]
            rz = wc[:, :, 0:128].transpose(1, 0, 2).reshape(128, G * 128)
            cc = wc[:, :, 128:192].transpose(1, 0, 2).reshape(128, G * 64)
            blocks.append(
                np.ascontiguousarray(
                    np.concatenate([rz, cc], axis=1)).reshape(-1))
            s += G
        maps.append({"xhx": xhx, "auxf": auxf,
                     "wxh": np.concatenate(blocks)})
    return maps


def kernel(x, state, Wx, Wh, b, _trace=False):
    nc = _get_nc()
    in_maps = _pack(x, state, Wx, Wh, b)
    res = run_bass_kernel_spmd(nc, in_maps, list(range(NCORES)), trace=_trace)
    out = np.concatenate(
        [res.results[i]["out"].T for i in range(NCORES)], axis=0)
    ret = np.ascontiguousarray(out.reshape(B, N, 1, H), dtype=np.float32)
    if _trace:
        return ret, res
    return ret


# revision 4
# speedup vs baseline: 3.8463x; 1.1453x over previous
"""Trainium2 Bass kernel for batched per-item GRU cell (bf16 PE pipeline).

Problem: nn_GRU_Cell — B=16, N=207 independent items, each with its own
C=64 -> 3H=192 weight matrices (Wx, Wh).  All ops are per-(b,n):

    xW          = x @ Wx                      [1, 192]
    r           = sigmoid(xW_r + h @ Wh_r + b_r)
    z           = sigmoid(xW_z + h @ Wh_z + b_z)
    hc          = tanh  (xW_c + (r*h) @ Wh_c + b_c)
    h_new       = (1 - z) * h + z * hc

Strategy (per core, items sharded 3312 -> 8 x 414):
  * Weights are both the DMA and the PE bottleneck.  They stream ONCE as
    bf16 (halves HBM traffic vs f32, and bf16 stationaries load with the
    PE's Fast-Weight-Load at ~2x; f32 matmuls would also double-pass).
    Accuracy: bf16 weights/activations with f32 PSUM accumulate gives
    rel-err ~1.8e-3 on this problem (gate is 2e-2).
  * Per item the weights are the PE *stationary* operand, K-stacked:
      S_rz       = [Wx[:, 0:128] ; Wh[:, 0:128]]    (K=128, M=128)
      S_c (pair) = [Wc_even | Wc_odd], Wc = [Wx[:,128:192]; Wh[:,128:192]]
    The c-weights of two adjacent items are packed into ONE 128-column
    stationary so every LDWEIGHTS is a full 128-column load (FWL-eligible)
    and two c-matmuls share one weight load.
  * Moving operands are single bf16 columns:
      rz: [x ; h]     -> psum_rz[:, g]  (r rows 0:64, z rows 64:128)
      c : [x ; r*h]   -> psum_c [:, g]  (even items' c in rows 0:64,
                                         odd items' c in rows 64:128)
  * All input transposition is done HOST-side: x/h/biases arrive as
    [feature, item] panels, so the kernel issues ZERO PE transposes and
    no ACT staging copies.  The output stays [H, items]; the host
    transposes it back.
  * The c-pass of chunk k is issued after the rz-pass of chunk k+1, so
    the PE never waits on the sigmoid/DVE producing the c moving operand.
"""

import numpy as np

import concourse.bass as bass
import concourse.mybir as mybir
import concourse.tile as tile
from concourse import bacc
from concourse.bass_utils import run_bass_kernel_spmd

F32 = mybir.dt.float32
BF16 = mybir.dt.bfloat16
AF = mybir.ActivationFunctionType

B, N, C, H = 16, 207, 64, 64
J = 3 * H                  # 192
ITEMS = B * N              # 3312
NCORES = 8
PER = ITEMS // NCORES      # 414
# Small first chunk so the PE starts (and buffer recycling begins) early;
# small last chunks so the post-DMA tail is short.  All even (c-pass pairs).
CHUNKS = [16] + [52] * 7 + [20] + [14]   # sum = 414
NCHUNK = len(CHUNKS)
GMAX = max(CHUNKS)


def build_nc():
    nc = bacc.Bacc(None)
    # host-pre-transposed activation panels
    #   xhx  bf16 [128, 2*PER]: cols 0:PER = [x;h], cols PER:2PER = [x;0]
    #        (rows 64:128 of the second block get r*h written on-chip)
    #   auxf f32  [128, 3*PER]: block0 rows 64:128 = h, block1 = b_rz^T,
    #        block2 = b_c duplicated (even cols rows 0:64, odd rows 64:128)
    xhx_d = nc.declare_dram_parameter("xhx", [128, 2 * PER], BF16,
                                      isOutput=False)
    auxf_d = nc.declare_dram_parameter("auxf", [128, 3 * PER], F32,
                                       isOutput=False)
    # per-chunk blocks, each [c=128, G*192] flattened: G rz-stationaries
    # (128 cols each) then G/2 paired c-stationaries (128 cols each)
    w_d = nc.declare_dram_parameter("wxh", [PER * 2 * C * J], BF16,
                                    isOutput=False)
    out_d = nc.declare_dram_parameter("out", [64, PER], F32, isOutput=True)

    with tile.TileContext(nc) as tc:
        with (
            tc.tile_pool(name="const", bufs=1) as cpool,
            tc.tile_pool(name="w", bufs=6) as wpool,
            tc.tile_pool(name="ep", bufs=2) as epool,
            tc.tile_pool(name="prz", bufs=3, space="PSUM") as prz_pool,
            tc.tile_pool(name="pc", bufs=3, space="PSUM") as pc_pool,
        ):
            # preloads go on the SWDGE queue so both HWDGE queues carry
            # nothing but the back-to-back weight stream
            xhx = cpool.tile([128, 2 * PER], BF16)
            nc.gpsimd.dma_start(out=xhx[:], in_=xhx_d[:])
            auxf = cpool.tile([128, 3 * PER], F32)
            nc.gpsimd.dma_start(out=auxf[:], in_=auxf_d[:])
            hn = cpool.tile([128, PER], F32)

            h_f = auxf[:, 0:PER]               # rows 64:128 = h (f32)
            b_rz = auxf[:, PER:2 * PER]
            bc2 = auxf[:, 2 * PER:3 * PER]
            rhs2 = xhx[:, PER:2 * PER]         # rows 0:64 = x (bf16)

            # software pipeline state: chunk k's c-pass+epilogue runs after
            # chunk k+1's rz-pass so PE never stalls on the sigmoid chain
            pending = None

            def rz_pass(k, s, G):
                w = wpool.tile([128, GMAX * J], BF16, tag="w")
                wq = nc.sync if k % 2 == 0 else nc.scalar
                wq.dma_start(
                    out=w[:, 0:G * J],
                    in_=w_d[s * 128 * J:(s + G) * 128 * J].rearrange(
                        "(c v) -> c v", c=128),
                )
                psum_rz = prz_pool.tile([128, GMAX], F32, tag="rz")
                for g in range(G):
                    nc.tensor.matmul(
                        psum_rz[:, g:g + 1],
                        w[:, g * 128:(g + 1) * 128],
                        xhx[:, s + g:s + g + 1],
                        start=True, stop=True,
                    )
                # r/z and the c-pass moving columns
                t_rz = epool.tile([128, GMAX], F32, tag="t_rz")
                nc.vector.tensor_add(t_rz[:, 0:G], psum_rz[:, 0:G],
                                     b_rz[:, s:s + G])
                rs = epool.tile([128, GMAX], F32, tag="rs")
                nc.scalar.activation(rs[64:128, 0:G], t_rz[0:64, 0:G],
                                     AF.Sigmoid)
                zs = epool.tile([128, GMAX], F32, tag="zs")
                nc.scalar.activation(zs[64:128, 0:G], t_rz[64:128, 0:G],
                                     AF.Sigmoid)
                nc.vector.tensor_mul(rhs2[64:128, s:s + G], rs[64:128, 0:G],
                                     h_f[64:128, s:s + G])
                return w, zs

            def c_pass(k, s, G, w, zs):
                psum_c = pc_pool.tile([128, GMAX], F32, tag="c")
                cbase = G * 128
                for t in range(G // 2):
                    lw = w[:, cbase + t * 128:cbase + (t + 1) * 128]
                    nc.tensor.matmul(
                        psum_c[:, 2 * t:2 * t + 1], lw,
                        rhs2[:, s + 2 * t:s + 2 * t + 1],
                        start=True, stop=True,
                    )
                    nc.tensor.matmul(
                        psum_c[:, 2 * t + 1:2 * t + 2], lw,
                        rhs2[:, s + 2 * t + 1:s + 2 * t + 2],
                        start=True, stop=True,
                    )
                t_c = epool.tile([128, GMAX], F32, tag="t_c")
                nc.vector.tensor_add(t_c[:, 0:G], psum_c[:, 0:G],
                                     bc2[:, s:s + G])
                # even items' c sits in rows 0:64, odd items' in 64:128
                hc = epool.tile([128, GMAX], F32, tag="hc")
                nc.scalar.activation(hc[64:128, 0:G:2], t_c[0:64, 0:G:2],
                                     AF.Tanh)
                nc.scalar.activation(hc[64:128, 1:G:2], t_c[64:128, 1:G:2],
                                     AF.Tanh)
                # h_new = h + z*(hc - h)
                diff = epool.tile([128, GMAX], F32, tag="diff")
                nc.vector.tensor_sub(diff[64:128, 0:G], hc[64:128, 0:G],
                                     h_f[64:128, s:s + G])
                prod = epool.tile([128, GMAX], F32, tag="prod")
                nc.vector.tensor_mul(prod[64:128, 0:G], zs[64:128, 0:G],
                                     diff[64:128, 0:G])
                nc.vector.tensor_add(hn[64:128, s:s + G],
                                     h_f[64:128, s:s + G],
                                     prod[64:128, 0:G])
                nc.gpsimd.dma_start(out=out_d[:, s:s + G],
                                    in_=hn[64:128, s:s + G])

            s = 0
            for k in range(NCHUNK):
                G = CHUNKS[k]
                state = rz_pass(k, s, G)
                if pending is not None:
                    c_pass(*pending)
                pending = (k, s, G) + state
                s += G
            c_pass(*pending)

    nc.compile()
    return nc


_CACHE = {}


def _get_nc():
    if "nc" not in _CACHE:
        _CACHE["nc"] = build_nc()
    return _CACHE["nc"]


def _pack(x, state, Wx, Wh, b):
    import ml_dtypes
    BF = ml_dtypes.bfloat16
    x2 = np.asarray(x, np.float32).reshape(ITEMS, C)
    h2 = np.asarray(state, np.float32).reshape(ITEMS, H)
    b2 = np.asarray(b, np.float32).reshape(ITEMS, J)
    wx = np.asarray(Wx, np.float32).reshape(ITEMS, C, J)
    wh = np.asarray(Wh, np.float32).reshape(ITEMS, H, J)
    w2 = np.concatenate([wx, wh], axis=1).astype(BF)   # [ITEMS, 128, 192]
    maps = []
    for i in range(NCORES):
        sl = slice(i * PER, (i + 1) * PER)
        xi, hi, bi, wi = x2[sl], h2[sl], b2[sl], w2[sl]

        xhx = np.zeros((128, 2 * PER), BF)
        xhx[0:64, 0:PER] = xi.T
        xhx[64:128, 0:PER] = hi.T
        xhx[0:64, PER:2 * PER] = xi.T

        auxf = np.zeros((128, 3 * PER), np.float32)
        auxf[64:128, 0:PER] = hi.T
        auxf[:, PER:2 * PER] = bi[:, 0:128].T
        bc = bi[:, 128:192].T                    # [64, PER]
        auxf[0:64, 2 * PER + 0:3 * PER:2] = bc[:, 0::2]
        auxf[64:128, 2 * PER + 1:3 * PER:2] = bc[:, 1::2]

        blocks = []
        s = 0
        for G in CHUNKS:
            wc = wi[s:s + G]                               # [G, 128, 192]
            rz = wc[:, :, 0:128].transpose(1, 0, 2).reshape(128, G * 128)
            cc = wc[:, :, 128:192].transpose(1, 0, 2).reshape(128, G * 64)
            blocks.append(
                np.ascontiguousarray(
                    np.concatenate([rz, cc], axis=1)).reshape(-1))
            s += G
        maps.append({"xhx": xhx, "auxf": auxf,
                     "wxh": np.concatenate(blocks)})
    return maps


def kernel(x, state, Wx, Wh, b, _trace=False):
    nc = _get_nc()
    in_maps = _pack(x, state, Wx, Wh, b)
    res = run_bass_kernel_spmd(nc, in_maps, list(range(NCORES)), trace=_trace)
    out = np.concatenate(
        [res.results[i]["out"].T for i in range(NCORES)], axis=0)
    ret = np.ascontiguousarray(out.reshape(B, N, 1, H), dtype=np.float32)
    if _trace:
        return ret, res
    return ret
